# revision 65
# baseline (speedup 1.0000x reference)
"""Multi-head attention (B=4, N=2048, DIM=768, H=12) on 8 TRN2 cores, bf16.

Sharding: core c -> batch c//2, heads (c%2)*6 .. +6  (6 heads = 3 pairs).
Each core computes its heads' attention and a partial output projection
(row-sharded w_proj); host sums the two partials per batch and adds bias.

Per-core dataflow:
  inputs : host pre-packs every dram param into its exact SBUF layout so
           each loads with one plain 2D DMA; wk and xtq0 come first (and
           split in halves) so the first matmul starts ~1MB into the DMA
           stream. A dep-free junk-matmul burst warms the PE HAM clock
           gate (1.2 -> 2.4 GHz) while the inputs stream in.
  qkv    : Q^T,K^T per head-pair [128,2048] (d-major), V token-major with
           a ones column per head ([64 V | 1] x 6 -> [128, 390])
  scores : S^T[keys, q] per key tile, both heads row-tiled (concurrent in
           the top/bottom array halves); exp on ACT with scale folded in
           (max-subtraction skipped - scores are O(5))
  PV     : U accumulated over key tiles; the ones column makes row 64 the
           softmax denominator for free
  norm   : den rows -> SBUF, reciprocal_approx_fast, bf16 cast, bf16 PE
           ones-matmul replication, head B shifted to partitions 64:128
           via identity matmul, one multiply per head
  proj   : partial = OT.T-slices @ wp (OT is d-major already)

Schedule: one flat software-pipelined stream over all 192 (pair, qb,
key-tile) units -- scores/exp runs LEAD=2 slots ahead of PV and flows
straight across qb/pair boundaries so the exp engine (the cadence
setter, ~1.15us per [128,1024] tile) never waits for a loop turnaround.
V' is built just-in-time inside the first loop; the remaining qkv
projections and the output projection ride the stream as 3-matmul
filler bursts placed so every deadline is met in trace order; each qb's
normalization tail is split (DVE part, then PE part a few slots later)
so the in-order PE queue is never parked on it. The epilogue prefills
the last proj groups' first two pair-accumulations into freed psum
banks while the final tail's DVE chain runs.
"""

import sys

for _p in ("/opt/trn_rl_repo",):
    if _p not in sys.path:
        sys.path.insert(0, _p)

import numpy as np
import ml_dtypes

import concourse.bass as bass
import concourse.bacc as bacc
import concourse.mybir as mybir
import concourse.tile as tile
from concourse.bass_utils import run_bass_kernel_spmd

DIM = 768
HEADS = 12
HD = 64
B = 4
N = 2048
NCORES = 8
PAIRS = 3          # head-pairs per core (6 heads)
CH = DIM // 128    # 6 contraction chunks of 128
KT = N // 128      # 16 key tiles
QB = N // 512      # 4 query blocks of 512
F32 = mybir.dt.float32
EXP = mybir.ActivationFunctionType.Exp
SCALE = HD ** -0.5

DEFAULT_DTYPE = "bf16"


def build_program(dtype="bf16", overlap=None):
    if overlap is None:
        overlap = True
    dt = F32 if dtype == "f32" else mybir.dt.bfloat16
    nc = bacc.Bacc()
    # host pre-blocks everything into the exact SBUF layouts (one DMA each):
    #   xtq[qb] : [128, CH*512]   xtq[qb][p, ch*512+c] = x.T[ch*128+p, qb*512+c]
    #   wqkv    : [128, CH*1152]  [wq(384) | wk(384) | wv(384)] per chunk
    #   wp      : [128, 3*768]    wp[p, ch*768+e] = w_proj[h0+ch*128+p, e]
    xtq = nc.declare_dram_parameter("xtq", [QB * 128, CH * 512], dt, isOutput=False)
    wqkv = nc.declare_dram_parameter("wqkv", [128, CH * 1152], dt, isOutput=False)
    wp = nc.declare_dram_parameter("wp", [128, PAIRS * 768], dt, isOutput=False)
    out = nc.declare_dram_parameter("out", [N, DIM], F32, isOutput=True)

    with tile.TileContext(nc) as tc:
        emit(tc, nc, xtq, wqkv, wp, out, dt, overlap=overlap)
    nc.compile()
    return nc


def emit(tc, nc, xtq, wqkv, wp, out, dt, overlap):
    import contextlib

    ctx = contextlib.ExitStack()
    wbufs = 2 if overlap else 1
    with ctx:
        sb = ctx.enter_context(tc.tile_pool(name="sb", bufs=1))
        ps = ctx.enter_context(tc.tile_pool(name="ps", bufs=1, space="PSUM"))

        # ---- load inputs (8 DMAs, ordered by first use) ------------------
        # wqkv dram layout: [wk_pack | wq_pack | wv_pack], each [128, CH*384]
        # with pack[p, ch*384+j] = w[ch*128+p, j]. wk loads first (the K
        # projection is the first compute); xtq tiles stream in qb order.
        # wk and xtq0 feed the very first matmuls: they load in chunk-halves
        # (separate tiles) so chunk 0's matmul starts after ~1MB of DMA.
        wk_sb2 = [sb.tile([128, 3 * 384], dt, name=f"wk{h}", tag=f"wk{h}")
                  for h in range(2)]
        wq_sb = sb.tile([128, CH * 384], dt, name="wq", tag="wq")
        wv_sb = sb.tile([128, CH * 384], dt, name="wv", tag="wv")
        W = CH * 384
        xtq0_sb2 = [sb.tile([128, 3 * 512], dt, name=f"xtq0{h}", tag=f"xtq0{h}")
                    for h in range(2)]
        xtq_sb = [None] + [
            sb.tile([128, CH * 512], dt, name=f"xtq{qb}", tag=f"xtq{qb}")
            for qb in range(1, QB)
        ]
        nc.sync.dma_start(out=wk_sb2[0][:], in_=wqkv[:, 0:1152])
        nc.sync.dma_start(out=xtq0_sb2[0][:], in_=xtq[0:128, 0:1536])
        nc.sync.dma_start(out=wk_sb2[1][:], in_=wqkv[:, 1152:2304])
        nc.sync.dma_start(out=xtq0_sb2[1][:], in_=xtq[0:128, 1536:3072])
        nc.sync.dma_start(out=wq_sb[:], in_=wqkv[:, W:2 * W])
        nc.sync.dma_start(out=wv_sb[:], in_=wqkv[:, 2 * W:3 * W])
        for qb in range(1, QB):
            nc.sync.dma_start(out=xtq_sb[qb][:], in_=xtq[qb * 128:(qb + 1) * 128, :])
        wp_sb = sb.tile([128, PAIRS * 768], dt, name="wp", tag="wp")
        nc.sync.dma_start(out=wp_sb[:], in_=wp[:, :])

        def xt_ap(ch, c0, c1):
            """Columns [c0, c1) of x.T chunk ch; must lie in one qb block."""
            qb = c0 // 512
            assert (c1 - 1) // 512 == qb
            if qb == 0:
                return xtq0_sb2[ch // 3][:, (ch % 3) * 512 + c0:(ch % 3) * 512 + c1]
            base = ch * 512 + c0 - qb * 512
            return xtq_sb[qb][:, base:base + (c1 - c0)]

        def wq_ap(ch, p):
            return wq_sb[:, ch * 384 + p * 128: ch * 384 + (p + 1) * 128]

        def wk_ap(ch, p):
            return wk_sb2[ch // 3][:, (ch % 3) * 384 + p * 128:(ch % 3) * 384 + (p + 1) * 128]

        def wv_ap(ch):
            return wv_sb[:, ch * 384:(ch + 1) * 384]

        ones_sb = sb.tile([1, 64], dt, name="ones", tag="ones")
        nc.vector.memset(ones_sb[:], 1.0)
        ident = sb.tile([64, 64], dt, name="ident", tag="ident")
        from concourse.masks import make_identity
        make_identity(nc, ident)

        # HAM warm-up: dep-free junk matmuls right at program start so the
        # PE clock-gate opens (1.2 -> 2.4 GHz) while the input DMAs stream.
        warm = ps.tile([128, 512], F32, name="warm", tag="dr", bufs=2)
        for _ in range(80):
            nc.tensor.matmul(
                warm[0:64, 0:64], lhsT=ident[:, :], rhs=ident[:, :],
                start=True, stop=True,
            )

        # persistent SBUF tensors
        # v' layout per head g: cols [g*65 .. g*65+63] = V, col g*65+64 = 1.0
        v_sb = [sb.tile([128, 6 * 65], dt, name=f"v{k}", tag=f"v{k}")
                for k in range(KT)]
        ot_sb = [sb.tile([128, N], dt, name=f"ot{p}", tag=f"ot{p}")
                 for p in range(PAIRS)]

        # ---- V' (token-major, ones col per head) -------------------------
        def emit_v(kt):
            pv = ps.tile([128, 512], F32, name="dr", tag="dr", bufs=2)
            for ch in range(CH):
                nc.tensor.matmul(
                    pv[:, :PAIRS * 128],
                    lhsT=xt_ap(ch, kt * 128, (kt + 1) * 128),
                    rhs=wv_ap(ch),
                    start=(ch == 0), stop=(ch == CH - 1),
                )
            v3 = v_sb[kt].rearrange("p (g c) -> p g c", c=65)
            p3 = pv[:, :PAIRS * 128].rearrange("p (g c) -> p g c", c=64)
            nc.vector.tensor_copy(v3[:, :, 0:64], p3[:])
            nc.vector.memset(v3[:, :, 64:65], 1.0)

        qt_tiles = {}
        kt_tiles = {}
        qkv_acc = {}

        def emit_qkv_group(p, which, qb, half=None):
            """One accumulation group: 512 columns of Q^T or K^T for pair p.
            half=0/1 emits only the first/second 3-chunk burst (the pair of
            half-bursts must be emitted in order, at adjacent fill slots)."""
            w_ap, store, nm = (
                (wq_ap, qt_tiles, "qt") if which == 0 else (wk_ap, kt_tiles, "kt")
            )
            chs = range(CH) if half is None else range(3 * half, 3 * half + 3)
            if qb == 0 and half in (None, 0):
                store[p] = sb.tile([128, N], dt, name=f"{nm}{p}", tag=nm, bufs=wbufs)
            if half in (None, 0):
                qkv_acc[(p, which, qb)] = ps.tile(
                    [128, 512], F32, name="dr", tag="dr", bufs=2)
            acc = qkv_acc[(p, which, qb)]
            for ch in chs:
                nc.tensor.matmul(
                    acc[:],
                    lhsT=w_ap(ch, p),
                    rhs=xt_ap(ch, qb * 512, (qb + 1) * 512),
                    start=(ch == 0), stop=(ch == CH - 1),
                )
            if half in (None, 1):
                nc.vector.tensor_copy(store[p][:, qb * 512:(qb + 1) * 512], acc[:])
                del qkv_acc[(p, which, qb)]

        proj_acc = {}

        def emit_proj_group(tt, st_on_scalar=False, half=None):
            # lives in the dr pool (two banks) so it can never stall the
            # scores("s") psum rotation at qb boundaries. half=0 emits the
            # first 512 output columns' matmuls, half=1 the rest + writeout.
            tsl = slice(tt * 128, (tt + 1) * 128)
            if half in (None, 0):
                proj_acc[tt] = (
                    ps.tile([128, 512], F32, name="dr", tag="dr", bufs=2),
                    ps.tile([128, 512], F32, name="dr", tag="dr", bufs=2),
                )
            ppa, ppb = proj_acc[tt]
            if half in (None, 0):
                for ch in range(PAIRS):
                    nc.tensor.matmul(
                        ppa[:], lhsT=ot_sb[ch][:, tsl],
                        rhs=wp_sb[:, ch * 768:ch * 768 + 512],
                        start=(ch == 0), stop=(ch == PAIRS - 1),
                    )
            if half in (None, 1):
                for ch in range(PAIRS):
                    nc.tensor.matmul(
                        ppb[:, 0:256], lhsT=ot_sb[ch][:, tsl],
                        rhs=wp_sb[:, ch * 768 + 512:ch * 768 + 768],
                        start=(ch == 0), stop=(ch == PAIRS - 1),
                    )
                st = sb.tile([128, 768], F32, name="st", tag="st", bufs=2)
                if st_on_scalar:
                    # epilogue: exp work is done, the ACT engine is idle
                    nc.scalar.copy(st[:, 0:512], ppa[:])
                    nc.scalar.copy(st[:, 512:768], ppb[:, 0:256])
                else:
                    nc.vector.tensor_copy(st[:, 0:512], ppa[:])
                    nc.vector.tensor_copy(st[:, 512:768], ppb[:, 0:256])
                nc.sync.dma_start(out=out[tsl, :], in_=st[:])
                del proj_acc[tt]

        def emit_scores(p, qb, kt):
            """Scores + exp for one key tile; returns the e_sb tile."""
            qt_t = qt_tiles[p]
            kt_t = kt_tiles[p]
            qsl = slice(qb * 512, (qb + 1) * 512)
            ksl = slice(kt * 128, (kt + 1) * 128)
            s_ps = ps.tile([128, 1024], F32, name="s", tag="s", bufs=2)
            # scores S^T for both heads, row-tiled (contract=64 each)
            nc.tensor.matmul(
                s_ps[:, 0:512],
                lhsT=kt_t[0:64, ksl], rhs=qt_t[0:64, qsl],
                start=True, stop=True,
            )
            nc.tensor.matmul(
                s_ps[:, 512:1024],
                lhsT=kt_t[64:128, ksl], rhs=qt_t[64:128, qsl],
                start=True, stop=True,
            )
            e_sb = sb.tile([128, 1024], dt, name="e", tag="e", bufs=5)
            nc.scalar.activation(e_sb[:], s_ps[:], EXP, scale=SCALE)
            return e_sb

        def emit_pv(ustate, p, qb, kt, e_sb):
            u_a, u_b = ustate
            first = kt == 0
            last = kt == KT - 1
            # PV with the ones column: U[0:64] = P@V, U[64] = denominator
            nc.tensor.matmul(
                u_a[0:65, :],
                lhsT=v_sb[kt][:, (2 * p) * 65:(2 * p) * 65 + 65],
                rhs=e_sb[:, 0:512],
                start=first, stop=last,
            )
            nc.tensor.matmul(
                u_b[0:65, :],
                lhsT=v_sb[kt][:, (2 * p + 1) * 65:(2 * p + 1) * 65 + 65],
                rhs=e_sb[:, 512:1024],
                start=first, stop=last,
            )

        def make_tail(ustate, p, qb, final=False, repl_psum=None):
            """Two-stage normalization tail. Stage 1 (DVE only): U copies
            first so the u psum banks free fast, then fast reciprocals and
            the bf16 cast. Stage 2 (fires a few slots later so the PE
            stream is never parked on it): ones-matmul replication, identity
            shift, final multiplies. final=True orders reciprocals first --
            at the kernel end latency-to-rbf matters, not u-bank release."""
            u_a, u_b = ustate
            qsl = slice(qb * 512, (qb + 1) * 512)
            st = {}

            def tail_dve():
                dsb = sb.tile([1, 1024], F32, name="dsb", tag="dsb", bufs=1)
                rec = sb.tile([1, 1024], F32, name="rec", tag="rec", bufs=1)
                rbf = sb.tile([1, 1024], dt, name="rbf", tag="rbf", bufs=1)
                ua_sb = sb.tile([64, 512], dt, name="uasb", tag="uasb", bufs=2)
                tmp = sb.tile([64, 512], dt, name="tmp", tag="tmp", bufs=2)
                if final:
                    # exp work is done: the idle ACT engine does the copies
                    # in parallel with the DVE reciprocal chain.
                    nc.scalar.copy(dsb[0:1, 0:512], u_a[64:65, :])
                    nc.scalar.copy(dsb[0:1, 512:1024], u_b[64:65, :])
                    nc.scalar.copy(ua_sb[:], u_a[0:64, :])
                    nc.scalar.copy(tmp[:], u_b[0:64, :])
                else:
                    nc.vector.tensor_copy(dsb[0:1, 0:512], u_a[64:65, :])
                    nc.vector.tensor_copy(ua_sb[:], u_a[0:64, :])
                    nc.vector.tensor_copy(dsb[0:1, 512:1024], u_b[64:65, :])
                    nc.vector.tensor_copy(tmp[:], u_b[0:64, :])
                nc.vector.reciprocal_approx_fast(out=rec[0:1, 0:512], in_=dsb[0:1, 0:512])
                nc.vector.reciprocal_approx_fast(out=rec[0:1, 512:1024], in_=dsb[0:1, 512:1024])
                nc.vector.tensor_copy(rbf[:], rec[:])
                st.update(rbf=rbf, ua_sb=ua_sb, tmp=tmp)

            def tail_pe():
                rbf, ua_sb, tmp = st["rbf"], st["ua_sb"], st["tmp"]
                if repl_psum is not None:
                    r_ps, o2 = repl_psum
                else:
                    r_ps = ps.tile([128, 512], F32, name="dr", tag="dr", bufs=2)
                nc.tensor.matmul(
                    r_ps[0:64, :], lhsT=ones_sb[0:1, 0:64], rhs=rbf[0:1, 0:512],
                    start=True, stop=True,
                )
                nc.tensor.matmul(
                    r_ps[64:128, :], lhsT=ones_sb[0:1, 0:64], rhs=rbf[0:1, 512:1024],
                    start=True, stop=True,
                )
                # head B's U moves to partitions 64-127: PE shift via identity
                if repl_psum is None:
                    o2 = ps.tile([128, 512], F32, name="o2", tag="dr", bufs=2)
                nc.tensor.matmul(
                    o2[64:128, :], lhsT=ident[:, :], rhs=tmp[:],
                    start=True, stop=True,
                )
                rsb = sb.tile([64, 512], F32, name="rsb", tag="rsb", bufs=1)
                nc.vector.tensor_copy(rsb[:], r_ps[64:128, :])
                nc.vector.tensor_mul(ot_sb[p][0:64, qsl], ua_sb[:], r_ps[0:64, :])
                nc.vector.tensor_mul(ot_sb[p][64:128, qsl], o2[64:128, :], rsb[:])

            return tail_dve, tail_pe

        # ---- schedule ----------------------------------------------------
        Q, K = 0, 1

        def g(p, which, qb, half=None):
            return lambda: emit_qkv_group(p, which, qb, half=half)

        def pj(tt, half=None):
            return lambda: emit_proj_group(tt, half=half)

        def burst2(plan, slots, fn, *args):
            """Add fn's two half-bursts at kt slots s and s+1."""
            s = slots
            plan.setdefault(s, []).append(fn(*args, half=0))
            plan.setdefault(s + 1, []).append(fn(*args, half=1))

        def mkplan(spec):
            plan = {}
            for s, fn, args in spec:
                burst2(plan, s, fn, *args)
            return plan

        plans = {
            (0, 0): mkplan([(0, g, (0, K, 1)), (4, g, (0, K, 2)),
                            (8, g, (0, K, 3)), (12, g, (0, Q, 1))]),
            (0, 1): mkplan([(3, g, (0, Q, 2)), (7, g, (0, Q, 3)),
                            (11, g, (1, K, 0))]),
            (0, 2): mkplan([(3, g, (1, K, 1)), (7, g, (1, K, 2)),
                            (11, g, (1, K, 3))]),
            (0, 3): mkplan([(3, g, (1, Q, 0)), (7, g, (1, Q, 1)),
                            (11, g, (1, Q, 2))]),
            (1, 0): mkplan([(5, g, (1, Q, 3))]),
            (1, 1): mkplan([(3, g, (2, K, 0)), (9, g, (2, K, 1))]),
            (1, 2): mkplan([(3, g, (2, K, 2)), (9, g, (2, K, 3))]),
            (1, 3): mkplan([(3, g, (2, Q, 0)), (9, g, (2, Q, 1))]),
            (2, 0): mkplan([(3, g, (2, Q, 2)), (9, g, (2, Q, 3))]),
            (2, 1): mkplan([(8, pj, (0,)), (10, pj, (1,)),
                            (12, pj, (2,)), (14, pj, (3,))]),
            (2, 2): mkplan([(8, pj, (4,)), (10, pj, (5,)),
                            (12, pj, (6,)), (14, pj, (7,))]),
            (2, 3): mkplan([(8, pj, (8,)), (10, pj, (9,)),
                            (12, pj, (10,)), (14, pj, (11,))]),
        }

        if overlap:
            # One flat software-pipelined stream over all 192 (p,qb,kt)
            # units: the scores/exp unit runs LEAD slots ahead of the PV
            # unit, flowing straight across qb/p boundaries so the ACT
            # engine never waits for a loop turnaround. Each qb's
            # normalization tail is deferred 2+LEAD slots so the PE queue
            # reaches the next scores first.
            LEAD = 2
            flat = [(p, qb, kt) for p in range(PAIRS) for qb in range(QB)
                    for kt in range(KT)]
            emit_qkv_group(0, K, 0)
            emit_qkv_group(0, Q, 0)
            ustates = {}
            equeue = {}
            pending = {}  # slot -> [closures]
            for i in range(LEAD):
                p, qb, kt = flat[i]
                equeue[i] = emit_scores(p, qb, kt)
            emit_v(0)
            emit_v(1)
            for i, (p, qb, kt) in enumerate(flat):
                if i + LEAD < len(flat):
                    p2, qb2, kt2 = flat[i + LEAD]
                    equeue[i + LEAD] = emit_scores(p2, qb2, kt2)
                if (p, qb) not in ustates:
                    ustates[(p, qb)] = (
                        ps.tile([128, 512], F32, name="ua", tag="u", bufs=2),
                        ps.tile([128, 512], F32, name="ub", tag="u", bufs=2),
                    )
                if p == 0 and qb == 0 and kt < KT - 2:
                    emit_v(kt + 2)
                emit_pv(ustates[(p, qb)], p, qb, kt, equeue.pop(i))
                if kt == KT - 1 and (p, qb) != (PAIRS - 1, QB - 1):
                    t_dve, t_pe = make_tail(ustates.pop((p, qb)), p, qb)
                    pending.setdefault(i + 1 + LEAD, []).append(t_dve)
                    pending.setdefault(i + 6 + LEAD, []).append(t_pe)
                for f in pending.pop(i, []):
                    f()
                for f in plans.get((p, qb), {}).get(kt, []):
                    f()
            for fs in sorted(pending):
                for f in pending[fs]:
                    f()
            # epilogue: while the final tail's DVE chain runs, prefill the
            # last four proj groups' first two pair-accumulations into the
            # now-free u/dr/s psum banks; after the final multiplies only
            # one 128-contraction matmul pair per group remains.
            def ppair(tag):
                if tag == "s":
                    s_t = ps.tile([128, 1024], F32, name="s", tag="s", bufs=2)
                    return (s_t[:, 0:512], s_t[:, 512:1024])
                return (
                    ps.tile([128, 512], F32, name=tag, tag=tag, bufs=2),
                    ps.tile([128, 512], F32, name=tag, tag=tag, bufs=2),
                )

            repl_s = ppair("s")
            t_dve, t_pe = make_tail(
                ustates.pop((PAIRS - 1, QB - 1)), PAIRS - 1, QB - 1,
                final=True, repl_psum=repl_s)
            t_dve()
            pairs = {13: ppair("dr"), 14: ppair("s"), 12: ppair("u")}
            for tt in (13, 14, 12):
                tsl = slice(tt * 128, (tt + 1) * 128)
                ppa, ppb = pairs[tt]
                for ch in (0, 1):
                    nc.tensor.matmul(
                        ppa[:], lhsT=ot_sb[ch][:, tsl],
                        rhs=wp_sb[:, ch * 768:ch * 768 + 512],
                        start=(ch == 0), stop=False,
                    )
                    nc.tensor.matmul(
                        ppb[:, 0:256], lhsT=ot_sb[ch][:, tsl],
                        rhs=wp_sb[:, ch * 768 + 512:ch * 768 + 768],
                        start=(ch == 0), stop=False,
                    )
            t_pe()
            for tt in (12, 13, 14):
                tsl = slice(tt * 128, (tt + 1) * 128)
                ppa, ppb = pairs[tt]
                nc.tensor.matmul(
                    ppa[:], lhsT=ot_sb[2][:, tsl],
                    rhs=wp_sb[:, 2 * 768:2 * 768 + 512],
                    start=False, stop=True,
                )
                nc.tensor.matmul(
                    ppb[:, 0:256], lhsT=ot_sb[2][:, tsl],
                    rhs=wp_sb[:, 2 * 768 + 512:2 * 768 + 768],
                    start=False, stop=True,
                )
                stt = sb.tile([128, 768], F32, name="st", tag="st", bufs=2)
                nc.scalar.copy(stt[:, 0:512], ppa[:])
                nc.scalar.copy(stt[:, 512:768], ppb[:, 0:256])
                nc.sync.dma_start(out=out[tsl, :], in_=stt[:])
            emit_proj_group(15, st_on_scalar=True)
        else:
            for kt in range(KT):
                emit_v(kt)
            for p in range(PAIRS):
                for qb in range(QB):
                    for which in (Q, K):
                        emit_qkv_group(p, which, qb)
                for qb in range(QB):
                    us = (
                        ps.tile([128, 512], F32, name="ua", tag="u", bufs=2),
                        ps.tile([128, 512], F32, name="ub", tag="u", bufs=2),
                    )
                    for kt in range(KT):
                        e = emit_scores(p, qb, kt)
                        emit_pv(us, p, qb, kt, e)
                    t_dve, t_pe = make_tail(us, p, qb)
                    t_dve()
                    t_pe()
            for tt in range(KT):
                emit_proj_group(tt)


_NC = {}


def _get_nc(dtype, overlap=None):
    key = (dtype, overlap)
    if key not in _NC:
        _NC[key] = build_program(dtype, overlap=overlap)
    return _NC[key]


def make_in_maps(x, w_qkv, w_proj, dtype):
    np_dt = np.float32 if dtype == "f32" else ml_dtypes.bfloat16
    in_maps = []
    for c in range(NCORES):
        b = c // 2
        h0 = (c % 2) * 6 * HD
        xt = np.ascontiguousarray(x[b].T)                      # [768, 2048]
        # xtq[qb*128+p, ch*512+c] = xt[ch*128+p, qb*512+c]
        xtq = np.transpose(
            xt.reshape(CH, 128, QB, 512), (2, 1, 0, 3)
        ).reshape(QB * 128, CH * 512)
        def pack(w):  # [768, 384] -> [128, CH*384], [p, ch*384+j] = w[ch*128+p, j]
            return np.transpose(
                w.reshape(CH, 128, 384), (1, 0, 2)
            ).reshape(128, CH * 384)

        wq = w_qkv[:, h0:h0 + 384]
        wk = w_qkv[:, DIM + h0:DIM + h0 + 384]
        wv = w_qkv[:, 2 * DIM + h0:2 * DIM + h0 + 384]
        # wqkv dram layout: [wk_pack | wq_pack | wv_pack]
        wqkv = np.concatenate([pack(wk), pack(wq), pack(wv)], axis=1)
        # wp[p, ch*768+e] = w_proj[h0 + ch*128 + p, e]
        wpm = np.transpose(
            w_proj[h0:h0 + 384, :].reshape(PAIRS, 128, DIM), (1, 0, 2)
        ).reshape(128, PAIRS * DIM)
        in_maps.append({
            "xtq": np.ascontiguousarray(xtq).astype(np_dt),
            "wqkv": np.ascontiguousarray(wqkv).astype(np_dt),
            "wp": np.ascontiguousarray(wpm).astype(np_dt),
        })
    return in_maps


def run(x, w_qkv, w_proj, b_proj, trace=False, dtype=None, overlap=None):
    dtype = dtype or DEFAULT_DTYPE
    x = np.asarray(x, dtype=np.float32)
    w_qkv = np.asarray(w_qkv, dtype=np.float32)
    w_proj = np.asarray(w_proj, dtype=np.float32)
    b_proj = np.asarray(b_proj, dtype=np.float32)

    in_maps = make_in_maps(x, w_qkv, w_proj, dtype)
    res = run_bass_kernel_spmd(_get_nc(dtype, overlap), in_maps, list(range(NCORES)),
                               trace=trace)
    full = np.empty((B, N, DIM), dtype=np.float32)
    for b in range(B):
        full[b] = res.results[2 * b]["out"] + res.results[2 * b + 1]["out"] + b_proj
    return full, res


def kernel(x, w_qkv, w_proj, b_proj):
    full, _ = run(x, w_qkv, w_proj, b_proj, trace=False)
    return full


# revision 66
# speedup vs baseline: 1.0010x; 1.0010x over previous
"""Multi-head attention (B=4, N=2048, DIM=768, H=12) on 8 TRN2 cores, bf16.

Sharding: core c -> batch c//2, heads (c%2)*6 .. +6  (6 heads = 3 pairs).
Each core computes its heads' attention and a partial output projection
(row-sharded w_proj); host sums the two partials per batch and adds bias.

Per-core dataflow:
  inputs : host pre-packs every dram param into its exact SBUF layout so
           each loads with one plain 2D DMA; wk and xtq0 come first (and
           split in halves) so the first matmul starts ~1MB into the DMA
           stream. A dep-free junk-matmul burst warms the PE HAM clock
           gate (1.2 -> 2.4 GHz) while the inputs stream in.
  qkv    : Q^T,K^T per head-pair [128,2048] (d-major), V token-major with
           a ones column per head ([64 V | 1] x 6 -> [128, 390])
  scores : S^T[keys, q] per key tile, both heads row-tiled (concurrent in
           the top/bottom array halves); exp on ACT with scale folded in
           (max-subtraction skipped - scores are O(5))
  PV     : U accumulated over key tiles; the ones column makes row 64 the
           softmax denominator for free
  norm   : den rows -> SBUF, reciprocal_approx_fast, bf16 cast, bf16 PE
           ones-matmul replication, head B shifted to partitions 64:128
           via identity matmul, one multiply per head
  proj   : partial = OT.T-slices @ wp (OT is d-major already)

Schedule: one flat software-pipelined stream over all 192 (pair, qb,
key-tile) units -- scores/exp runs LEAD=2 slots ahead of PV and flows
straight across qb/pair boundaries so the exp engine (the cadence
setter, ~1.15us per [128,1024] tile) never waits for a loop turnaround.
V' is built just-in-time inside the first loop; the remaining qkv
projections and the output projection ride the stream as 3-matmul
filler bursts placed so every deadline is met in trace order; each qb's
normalization tail is split (DVE part, then PE part a few slots later)
so the in-order PE queue is never parked on it. The epilogue prefills
the last proj groups' first two pair-accumulations into freed psum
banks while the final tail's DVE chain runs.
"""

import sys

for _p in ("/opt/trn_rl_repo",):
    if _p not in sys.path:
        sys.path.insert(0, _p)

import numpy as np
import ml_dtypes

import concourse.bass as bass
import concourse.bacc as bacc
import concourse.mybir as mybir
import concourse.tile as tile
from concourse.bass_utils import run_bass_kernel_spmd

DIM = 768
HEADS = 12
HD = 64
B = 4
N = 2048
NCORES = 8
PAIRS = 3          # head-pairs per core (6 heads)
CH = DIM // 128    # 6 contraction chunks of 128
KT = N // 128      # 16 key tiles
QB = N // 512      # 4 query blocks of 512
F32 = mybir.dt.float32
EXP = mybir.ActivationFunctionType.Exp
SCALE = HD ** -0.5

DEFAULT_DTYPE = "bf16"


def build_program(dtype="bf16", overlap=None):
    if overlap is None:
        overlap = True
    dt = F32 if dtype == "f32" else mybir.dt.bfloat16
    nc = bacc.Bacc()
    # host pre-blocks everything into the exact SBUF layouts (one DMA each):
    #   xtq[qb] : [128, CH*512]   xtq[qb][p, ch*512+c] = x.T[ch*128+p, qb*512+c]
    #   wqkv    : [128, CH*1152]  [wq(384) | wk(384) | wv(384)] per chunk
    #   wp      : [128, 3*768]    wp[p, ch*768+e] = w_proj[h0+ch*128+p, e]
    xtq = nc.declare_dram_parameter("xtq", [QB * 128, CH * 512], dt, isOutput=False)
    wqkv = nc.declare_dram_parameter("wqkv", [128, CH * 1152], dt, isOutput=False)
    wp = nc.declare_dram_parameter("wp", [128, PAIRS * 768], dt, isOutput=False)
    out = nc.declare_dram_parameter("out", [N, DIM], F32, isOutput=True)

    with tile.TileContext(nc) as tc:
        emit(tc, nc, xtq, wqkv, wp, out, dt, overlap=overlap)
    nc.compile()
    return nc


def emit(tc, nc, xtq, wqkv, wp, out, dt, overlap):
    import contextlib

    ctx = contextlib.ExitStack()
    wbufs = 2 if overlap else 1
    with ctx:
        sb = ctx.enter_context(tc.tile_pool(name="sb", bufs=1))
        ps = ctx.enter_context(tc.tile_pool(name="ps", bufs=1, space="PSUM"))

        # ---- load inputs (8 DMAs, ordered by first use) ------------------
        # wqkv dram layout: [wk_pack | wq_pack | wv_pack], each [128, CH*384]
        # with pack[p, ch*384+j] = w[ch*128+p, j]. wk loads first (the K
        # projection is the first compute); xtq tiles stream in qb order.
        # wk and xtq0 feed the very first matmuls: they load in chunk-halves
        # (separate tiles) so chunk 0's matmul starts after ~1MB of DMA.
        wk_sb2 = [sb.tile([128, 3 * 384], dt, name=f"wk{h}", tag=f"wk{h}")
                  for h in range(2)]
        wq_sb = sb.tile([128, CH * 384], dt, name="wq", tag="wq")
        wv_sb = sb.tile([128, CH * 384], dt, name="wv", tag="wv")
        W = CH * 384
        xtq0_sb2 = [sb.tile([128, 3 * 512], dt, name=f"xtq0{h}", tag=f"xtq0{h}")
                    for h in range(2)]
        xtq_sb = [None] + [
            sb.tile([128, CH * 512], dt, name=f"xtq{qb}", tag=f"xtq{qb}")
            for qb in range(1, QB)
        ]
        nc.sync.dma_start(out=wk_sb2[0][:], in_=wqkv[:, 0:1152])
        nc.sync.dma_start(out=xtq0_sb2[0][:], in_=xtq[0:128, 0:1536])
        nc.sync.dma_start(out=wk_sb2[1][:], in_=wqkv[:, 1152:2304])
        nc.sync.dma_start(out=xtq0_sb2[1][:], in_=xtq[0:128, 1536:3072])
        nc.sync.dma_start(out=wq_sb[:], in_=wqkv[:, W:2 * W])
        nc.sync.dma_start(out=wv_sb[:], in_=wqkv[:, 2 * W:3 * W])
        for qb in range(1, QB):
            nc.sync.dma_start(out=xtq_sb[qb][:], in_=xtq[qb * 128:(qb + 1) * 128, :])
        wp_sb = sb.tile([128, PAIRS * 768], dt, name="wp", tag="wp")
        nc.sync.dma_start(out=wp_sb[:], in_=wp[:, :])

        def xt_ap(ch, c0, c1):
            """Columns [c0, c1) of x.T chunk ch; must lie in one qb block."""
            qb = c0 // 512
            assert (c1 - 1) // 512 == qb
            if qb == 0:
                return xtq0_sb2[ch // 3][:, (ch % 3) * 512 + c0:(ch % 3) * 512 + c1]
            base = ch * 512 + c0 - qb * 512
            return xtq_sb[qb][:, base:base + (c1 - c0)]

        def wq_ap(ch, p):
            return wq_sb[:, ch * 384 + p * 128: ch * 384 + (p + 1) * 128]

        def wk_ap(ch, p):
            return wk_sb2[ch // 3][:, (ch % 3) * 384 + p * 128:(ch % 3) * 384 + (p + 1) * 128]

        def wv_ap(ch):
            return wv_sb[:, ch * 384:(ch + 1) * 384]

        ones_sb = sb.tile([1, 64], dt, name="ones", tag="ones")
        nc.vector.memset(ones_sb[:], 1.0)
        ident = sb.tile([64, 64], dt, name="ident", tag="ident")
        from concourse.masks import make_identity
        make_identity(nc, ident)

        # HAM warm-up: dep-free junk matmuls right at program start so the
        # PE clock-gate opens (1.2 -> 2.4 GHz) while the input DMAs stream.
        warm = ps.tile([128, 512], F32, name="warm", tag="dr", bufs=2)
        for _ in range(80):
            nc.tensor.matmul(
                warm[0:64, 0:64], lhsT=ident[:, :], rhs=ident[:, :],
                start=True, stop=True,
            )

        # persistent SBUF tensors
        # v' layout per head g: cols [g*65 .. g*65+63] = V, col g*65+64 = 1.0
        v_sb = [sb.tile([128, 6 * 65], dt, name=f"v{k}", tag=f"v{k}")
                for k in range(KT)]
        ot_sb = [sb.tile([128, N], dt, name=f"ot{p}", tag=f"ot{p}")
                 for p in range(PAIRS)]

        # ---- V' (token-major, ones col per head) -------------------------
        def emit_v(kt):
            pv = ps.tile([128, 512], F32, name="dr", tag="dr", bufs=2)
            for ch in range(CH):
                nc.tensor.matmul(
                    pv[:, :PAIRS * 128],
                    lhsT=xt_ap(ch, kt * 128, (kt + 1) * 128),
                    rhs=wv_ap(ch),
                    start=(ch == 0), stop=(ch == CH - 1),
                )
            v3 = v_sb[kt].rearrange("p (g c) -> p g c", c=65)
            p3 = pv[:, :PAIRS * 128].rearrange("p (g c) -> p g c", c=64)
            nc.vector.tensor_copy(v3[:, :, 0:64], p3[:])
            nc.vector.memset(v3[:, :, 64:65], 1.0)

        qt_tiles = {}
        kt_tiles = {}
        qkv_acc = {}

        def emit_qkv_group(p, which, qb, half=None):
            """One accumulation group: 512 columns of Q^T or K^T for pair p.
            half=0/1 emits only the first/second 3-chunk burst (the pair of
            half-bursts must be emitted in order, at adjacent fill slots)."""
            w_ap, store, nm = (
                (wq_ap, qt_tiles, "qt") if which == 0 else (wk_ap, kt_tiles, "kt")
            )
            chs = range(CH) if half is None else range(3 * half, 3 * half + 3)
            if qb == 0 and half in (None, 0):
                store[p] = sb.tile([128, N], dt, name=f"{nm}{p}", tag=nm, bufs=wbufs)
            if half in (None, 0):
                qkv_acc[(p, which, qb)] = ps.tile(
                    [128, 512], F32, name="dr", tag="dr", bufs=2)
            acc = qkv_acc[(p, which, qb)]
            for ch in chs:
                nc.tensor.matmul(
                    acc[:],
                    lhsT=w_ap(ch, p),
                    rhs=xt_ap(ch, qb * 512, (qb + 1) * 512),
                    start=(ch == 0), stop=(ch == CH - 1),
                )
            if half in (None, 1):
                nc.vector.tensor_copy(store[p][:, qb * 512:(qb + 1) * 512], acc[:])
                del qkv_acc[(p, which, qb)]

        proj_acc = {}

        def emit_proj_group(tt, st_on_scalar=False, half=None):
            # lives in the dr pool (two banks) so it can never stall the
            # scores("s") psum rotation at qb boundaries. half=0 emits the
            # first 512 output columns' matmuls, half=1 the rest + writeout.
            tsl = slice(tt * 128, (tt + 1) * 128)
            if half in (None, 0):
                proj_acc[tt] = (
                    ps.tile([128, 512], F32, name="dr", tag="dr", bufs=2),
                    ps.tile([128, 512], F32, name="dr", tag="dr", bufs=2),
                )
            ppa, ppb = proj_acc[tt]
            if half in (None, 0):
                for ch in range(PAIRS):
                    nc.tensor.matmul(
                        ppa[:], lhsT=ot_sb[ch][:, tsl],
                        rhs=wp_sb[:, ch * 768:ch * 768 + 512],
                        start=(ch == 0), stop=(ch == PAIRS - 1),
                    )
            if half in (None, 1):
                for ch in range(PAIRS):
                    nc.tensor.matmul(
                        ppb[:, 0:256], lhsT=ot_sb[ch][:, tsl],
                        rhs=wp_sb[:, ch * 768 + 512:ch * 768 + 768],
                        start=(ch == 0), stop=(ch == PAIRS - 1),
                    )
                st = sb.tile([128, 768], F32, name="st", tag="st", bufs=2)
                if st_on_scalar:
                    # epilogue: exp work is done, the ACT engine is idle
                    nc.scalar.copy(st[:, 0:512], ppa[:])
                    nc.scalar.copy(st[:, 512:768], ppb[:, 0:256])
                else:
                    nc.vector.tensor_copy(st[:, 0:512], ppa[:])
                    nc.vector.tensor_copy(st[:, 512:768], ppb[:, 0:256])
                nc.sync.dma_start(out=out[tsl, :], in_=st[:])
                del proj_acc[tt]

        def emit_scores(p, qb, kt):
            """Scores + exp for one key tile; returns the e_sb tile."""
            qt_t = qt_tiles[p]
            kt_t = kt_tiles[p]
            qsl = slice(qb * 512, (qb + 1) * 512)
            ksl = slice(kt * 128, (kt + 1) * 128)
            s_ps = ps.tile([128, 1024], F32, name="s", tag="s", bufs=2)
            # scores S^T for both heads, row-tiled (contract=64 each)
            nc.tensor.matmul(
                s_ps[:, 0:512],
                lhsT=kt_t[0:64, ksl], rhs=qt_t[0:64, qsl],
                start=True, stop=True,
            )
            nc.tensor.matmul(
                s_ps[:, 512:1024],
                lhsT=kt_t[64:128, ksl], rhs=qt_t[64:128, qsl],
                start=True, stop=True,
            )
            e_sb = sb.tile([128, 1024], dt, name="e", tag="e", bufs=5)
            nc.scalar.activation(e_sb[:], s_ps[:], EXP, scale=SCALE)
            return e_sb

        def emit_pv(ustate, p, qb, kt, e_sb):
            u_a, u_b = ustate
            first = kt == 0
            last = kt == KT - 1
            # PV with the ones column: U[0:64] = P@V, U[64] = denominator
            nc.tensor.matmul(
                u_a[0:65, :],
                lhsT=v_sb[kt][:, (2 * p) * 65:(2 * p) * 65 + 65],
                rhs=e_sb[:, 0:512],
                start=first, stop=last,
            )
            nc.tensor.matmul(
                u_b[0:65, :],
                lhsT=v_sb[kt][:, (2 * p + 1) * 65:(2 * p + 1) * 65 + 65],
                rhs=e_sb[:, 512:1024],
                start=first, stop=last,
            )

        def make_tail(ustate, p, qb, final=False, repl_psum=None):
            """Two-stage normalization tail. Stage 1 (DVE only): U copies
            first so the u psum banks free fast, then fast reciprocals and
            the bf16 cast. Stage 2 (fires a few slots later so the PE
            stream is never parked on it): ones-matmul replication, identity
            shift, final multiplies. final=True orders reciprocals first --
            at the kernel end latency-to-rbf matters, not u-bank release."""
            u_a, u_b = ustate
            qsl = slice(qb * 512, (qb + 1) * 512)
            st = {}

            def tail_dve():
                dsb = sb.tile([1, 1024], F32, name="dsb", tag="dsb", bufs=1)
                rec = sb.tile([1, 1024], F32, name="rec", tag="rec", bufs=1)
                rbf = sb.tile([1, 1024], dt, name="rbf", tag="rbf", bufs=1)
                ua_sb = sb.tile([64, 512], dt, name="uasb", tag="uasb", bufs=2)
                tmp = sb.tile([64, 512], dt, name="tmp", tag="tmp", bufs=2)
                if final:
                    # exp work is done: the idle ACT engine does the copies
                    # in parallel with the DVE reciprocal chain.
                    nc.scalar.copy(dsb[0:1, 0:512], u_a[64:65, :])
                    nc.scalar.copy(dsb[0:1, 512:1024], u_b[64:65, :])
                    nc.scalar.copy(ua_sb[:], u_a[0:64, :])
                    nc.scalar.copy(tmp[:], u_b[0:64, :])
                else:
                    nc.vector.tensor_copy(dsb[0:1, 0:512], u_a[64:65, :])
                    nc.vector.tensor_copy(ua_sb[:], u_a[0:64, :])
                    nc.vector.tensor_copy(dsb[0:1, 512:1024], u_b[64:65, :])
                    nc.vector.tensor_copy(tmp[:], u_b[0:64, :])
                nc.vector.reciprocal_approx_fast(out=rec[0:1, 0:512], in_=dsb[0:1, 0:512])
                nc.vector.reciprocal_approx_fast(out=rec[0:1, 512:1024], in_=dsb[0:1, 512:1024])
                nc.vector.tensor_copy(rbf[:], rec[:])
                st.update(rbf=rbf, ua_sb=ua_sb, tmp=tmp)

            def tail_pe():
                rbf, ua_sb, tmp = st["rbf"], st["ua_sb"], st["tmp"]
                if repl_psum is not None:
                    r_ps, o2 = repl_psum
                else:
                    r_ps = ps.tile([128, 512], F32, name="dr", tag="dr", bufs=2)
                nc.tensor.matmul(
                    r_ps[0:64, :], lhsT=ones_sb[0:1, 0:64], rhs=rbf[0:1, 0:512],
                    start=True, stop=True,
                )
                nc.tensor.matmul(
                    r_ps[64:128, :], lhsT=ones_sb[0:1, 0:64], rhs=rbf[0:1, 512:1024],
                    start=True, stop=True,
                )
                # head B's U moves to partitions 64-127: PE shift via identity
                if repl_psum is None:
                    o2 = ps.tile([128, 512], F32, name="o2", tag="dr", bufs=2)
                nc.tensor.matmul(
                    o2[64:128, :], lhsT=ident[:, :], rhs=tmp[:],
                    start=True, stop=True,
                )
                rsb = sb.tile([64, 512], F32, name="rsb", tag="rsb", bufs=1)
                nc.vector.tensor_copy(rsb[:], r_ps[64:128, :])
                nc.vector.tensor_mul(ot_sb[p][0:64, qsl], ua_sb[:], r_ps[0:64, :])
                nc.vector.tensor_mul(ot_sb[p][64:128, qsl], o2[64:128, :], rsb[:])

            return tail_dve, tail_pe

        # ---- schedule ----------------------------------------------------
        Q, K = 0, 1

        def g(p, which, qb, half=None):
            return lambda: emit_qkv_group(p, which, qb, half=half)

        def pj(tt, half=None):
            return lambda: emit_proj_group(tt, half=half)

        def burst2(plan, slots, fn, *args):
            """Add fn's two half-bursts at kt slots s and s+1."""
            s = slots
            plan.setdefault(s, []).append(fn(*args, half=0))
            plan.setdefault(s + 1, []).append(fn(*args, half=1))

        def mkplan(spec):
            plan = {}
            for s, fn, args in spec:
                burst2(plan, s, fn, *args)
            return plan

        plans = {
            (0, 0): mkplan([(0, g, (0, K, 1)), (4, g, (0, K, 2)),
                            (8, g, (0, K, 3)), (12, g, (0, Q, 1))]),
            (0, 1): mkplan([(3, g, (0, Q, 2)), (7, g, (0, Q, 3)),
                            (11, g, (1, K, 0))]),
            (0, 2): mkplan([(3, g, (1, K, 1)), (7, g, (1, K, 2)),
                            (11, g, (1, K, 3))]),
            (0, 3): mkplan([(3, g, (1, Q, 0)), (7, g, (1, Q, 1)),
                            (11, g, (1, Q, 2))]),
            (1, 0): mkplan([(5, g, (1, Q, 3))]),
            (1, 1): mkplan([(3, g, (2, K, 0)), (9, g, (2, K, 1))]),
            (1, 2): mkplan([(3, g, (2, K, 2)), (9, g, (2, K, 3))]),
            (1, 3): mkplan([(3, g, (2, Q, 0)), (9, g, (2, Q, 1))]),
            (2, 0): mkplan([(3, g, (2, Q, 2)), (9, g, (2, Q, 3))]),
            (2, 1): mkplan([(8, pj, (0,)), (10, pj, (1,)),
                            (12, pj, (2,)), (14, pj, (3,))]),
            (2, 2): mkplan([(8, pj, (4,)), (10, pj, (5,)),
                            (12, pj, (6,)), (14, pj, (7,))]),
            (2, 3): mkplan([(8, pj, (8,)), (10, pj, (9,)),
                            (12, pj, (10,)), (14, pj, (11,))]),
        }

        if overlap:
            # One flat software-pipelined stream over all 192 (p,qb,kt)
            # units: the scores/exp unit runs LEAD slots ahead of the PV
            # unit, flowing straight across qb/p boundaries so the ACT
            # engine never waits for a loop turnaround. Each qb's
            # normalization tail is deferred 2+LEAD slots so the PE queue
            # reaches the next scores first.
            LEAD = 2
            flat = [(p, qb, kt) for p in range(PAIRS) for qb in range(QB)
                    for kt in range(KT)]
            emit_qkv_group(0, K, 0)
            emit_qkv_group(0, Q, 0)
            ustates = {}
            equeue = {}
            pending = {}  # slot -> [closures]
            for i in range(LEAD):
                p, qb, kt = flat[i]
                equeue[i] = emit_scores(p, qb, kt)
            emit_v(0)
            emit_v(1)
            for i, (p, qb, kt) in enumerate(flat):
                if i + LEAD < len(flat):
                    p2, qb2, kt2 = flat[i + LEAD]
                    equeue[i + LEAD] = emit_scores(p2, qb2, kt2)
                if (p, qb) not in ustates:
                    ustates[(p, qb)] = (
                        ps.tile([128, 512], F32, name="ua", tag="u", bufs=2),
                        ps.tile([128, 512], F32, name="ub", tag="u", bufs=2),
                    )
                if p == 0 and qb == 0 and kt < KT - 2:
                    emit_v(kt + 2)
                emit_pv(ustates[(p, qb)], p, qb, kt, equeue.pop(i))
                if kt == KT - 1 and (p, qb) != (PAIRS - 1, QB - 1):
                    t_dve, t_pe = make_tail(ustates.pop((p, qb)), p, qb)
                    pending.setdefault(i + 1, []).append(t_dve)
                    pending.setdefault(i + 6 + LEAD, []).append(t_pe)
                for f in pending.pop(i, []):
                    f()
                for f in plans.get((p, qb), {}).get(kt, []):
                    f()
            for fs in sorted(pending):
                for f in pending[fs]:
                    f()
            # epilogue: while the final tail's DVE chain runs, prefill the
            # last four proj groups' first two pair-accumulations into the
            # now-free u/dr/s psum banks; after the final multiplies only
            # one 128-contraction matmul pair per group remains.
            def ppair(tag):
                if tag == "s":
                    s_t = ps.tile([128, 1024], F32, name="s", tag="s", bufs=2)
                    return (s_t[:, 0:512], s_t[:, 512:1024])
                return (
                    ps.tile([128, 512], F32, name=tag, tag=tag, bufs=2),
                    ps.tile([128, 512], F32, name=tag, tag=tag, bufs=2),
                )

            repl_s = ppair("s")
            t_dve, t_pe = make_tail(
                ustates.pop((PAIRS - 1, QB - 1)), PAIRS - 1, QB - 1,
                final=True, repl_psum=repl_s)
            t_dve()
            pairs = {13: ppair("dr"), 14: ppair("s"), 12: ppair("u")}
            for tt in (13, 14, 12):
                tsl = slice(tt * 128, (tt + 1) * 128)
                ppa, ppb = pairs[tt]
                for ch in (0, 1):
                    nc.tensor.matmul(
                        ppa[:], lhsT=ot_sb[ch][:, tsl],
                        rhs=wp_sb[:, ch * 768:ch * 768 + 512],
                        start=(ch == 0), stop=False,
                    )
                    nc.tensor.matmul(
                        ppb[:, 0:256], lhsT=ot_sb[ch][:, tsl],
                        rhs=wp_sb[:, ch * 768 + 512:ch * 768 + 768],
                        start=(ch == 0), stop=False,
                    )
            t_pe()
            for tt in (12, 13, 14):
                tsl = slice(tt * 128, (tt + 1) * 128)
                ppa, ppb = pairs[tt]
                nc.tensor.matmul(
                    ppa[:], lhsT=ot_sb[2][:, tsl],
                    rhs=wp_sb[:, 2 * 768:2 * 768 + 512],
                    start=False, stop=True,
                )
                nc.tensor.matmul(
                    ppb[:, 0:256], lhsT=ot_sb[2][:, tsl],
                    rhs=wp_sb[:, 2 * 768 + 512:2 * 768 + 768],
                    start=False, stop=True,
                )
                stt = sb.tile([128, 768], F32, name="st", tag="st", bufs=2)
                nc.scalar.copy(stt[:, 0:512], ppa[:])
                nc.scalar.copy(stt[:, 512:768], ppb[:, 0:256])
                nc.sync.dma_start(out=out[tsl, :], in_=stt[:])
            emit_proj_group(15, st_on_scalar=True)
        else:
            for kt in range(KT):
                emit_v(kt)
            for p in range(PAIRS):
                for qb in range(QB):
                    for which in (Q, K):
                        emit_qkv_group(p, which, qb)
                for qb in range(QB):
                    us = (
                        ps.tile([128, 512], F32, name="ua", tag="u", bufs=2),
                        ps.tile([128, 512], F32, name="ub", tag="u", bufs=2),
                    )
                    for kt in range(KT):
                        e = emit_scores(p, qb, kt)
                        emit_pv(us, p, qb, kt, e)
                    t_dve, t_pe = make_tail(us, p, qb)
                    t_dve()
                    t_pe()
            for tt in range(KT):
                emit_proj_group(tt)


_NC = {}


def _get_nc(dtype, overlap=None):
    key = (dtype, overlap)
    if key not in _NC:
        _NC[key] = build_program(dtype, overlap=overlap)
    return _NC[key]


def make_in_maps(x, w_qkv, w_proj, dtype):
    np_dt = np.float32 if dtype == "f32" else ml_dtypes.bfloat16
    in_maps = []
    for c in range(NCORES):
        b = c // 2
        h0 = (c % 2) * 6 * HD
        xt = np.ascontiguousarray(x[b].T)                      # [768, 2048]
        # xtq[qb*128+p, ch*512+c] = xt[ch*128+p, qb*512+c]
        xtq = np.transpose(
            xt.reshape(CH, 128, QB, 512), (2, 1, 0, 3)
        ).reshape(QB * 128, CH * 512)
        def pack(w):  # [768, 384] -> [128, CH*384], [p, ch*384+j] = w[ch*128+p, j]
            return np.transpose(
                w.reshape(CH, 128, 384), (1, 0, 2)
            ).reshape(128, CH * 384)

        wq = w_qkv[:, h0:h0 + 384]
        wk = w_qkv[:, DIM + h0:DIM + h0 + 384]
        wv = w_qkv[:, 2 * DIM + h0:2 * DIM + h0 + 384]
        # wqkv dram layout: [wk_pack | wq_pack | wv_pack]
        wqkv = np.concatenate([pack(wk), pack(wq), pack(wv)], axis=1)
        # wp[p, ch*768+e] = w_proj[h0 + ch*128 + p, e]
        wpm = np.transpose(
            w_proj[h0:h0 + 384, :].reshape(PAIRS, 128, DIM), (1, 0, 2)
        ).reshape(128, PAIRS * DIM)
        in_maps.append({
            "xtq": np.ascontiguousarray(xtq).astype(np_dt),
            "wqkv": np.ascontiguousarray(wqkv).astype(np_dt),
            "wp": np.ascontiguousarray(wpm).astype(np_dt),
        })
    return in_maps


def run(x, w_qkv, w_proj, b_proj, trace=False, dtype=None, overlap=None):
    dtype = dtype or DEFAULT_DTYPE
    x = np.asarray(x, dtype=np.float32)
    w_qkv = np.asarray(w_qkv, dtype=np.float32)
    w_proj = np.asarray(w_proj, dtype=np.float32)
    b_proj = np.asarray(b_proj, dtype=np.float32)

    in_maps = make_in_maps(x, w_qkv, w_proj, dtype)
    res = run_bass_kernel_spmd(_get_nc(dtype, overlap), in_maps, list(range(NCORES)),
                               trace=trace)
    full = np.empty((B, N, DIM), dtype=np.float32)
    for b in range(B):
        full[b] = res.results[2 * b]["out"] + res.results[2 * b + 1]["out"] + b_proj
    return full, res


def kernel(x, w_qkv, w_proj, b_proj):
    full, _ = run(x, w_qkv, w_proj, b_proj, trace=False)
    return full
